# revision 1
# baseline (speedup 1.0000x reference)
"""GCNConv kernel for 8x Trainium2 NeuronCores (Bass/Tile).

Reference computation:
    h = x @ W + b                  # [N, 256] @ [256, 128] -> [N, 128]
    out[i] = sum_{e: dst[e]=i} val[e] * h[src[e]]

Strategy (per core; SPMD - one program, per-core data):
  - dst nodes sharded 12500/core (output rows).  Edges partitioned by dst.
  - Bias folded in as a "virtual node": h[N] = b, plus one virtual edge
    (src=N, dst=i, val=sum of vals into i) per dst node.
  - Phase 1: every core computes the full h (fp16) into its own DRAM via
    PE matmuls (host passes x transposed + fp16).
  - Phase 2: per-edge h rows pulled on-chip with dma_gather (int16 indices,
    4 source windows of <=32767 rows).  Per 128-edge chunk, a host-built
    staircase matrix B [128e x 32seg] fp16 (carrying val) is the stationary
    matmul operand -> PSUM partial segment sums.  A second host-built
    one-hot S2 [128seg x 128dst] fp16 matmul accumulates segments into
    per-dst-tile PSUM, which is written out fp32.
All data-dependent structure is padded to the max across cores so the same
program serves all 8 cores.
"""

import sys

for _p in ("/opt/trn_rl_repo",):
    if _p not in sys.path:
        sys.path.insert(0, _p)

import numpy as np

P = 128
MSEG = 32            # segment slots per 128-edge chunk
TPG = 4              # dst tiles (of 128 dst) per processing group
N_CORES = 8


# ---------------------------------------------------------------------------
# Host-side preparation
# ---------------------------------------------------------------------------

def _ceil_to(a, m):
    return -(-a // m) * m


class Plan:
    """Static (core-invariant) program structure + per-core data arrays."""


def build_plan(x, edge_src, edge_dst, edge_vals, weight, bias):
    N, IN_F = x.shape
    OUT_F = weight.shape[1]
    assert N % N_CORES == 0
    ndst = N // N_CORES                    # dst nodes per core
    ndst_pad = _ceil_to(ndst, P)
    ntile = ndst_pad // P                  # dst tiles per core
    ngrp = -(-ntile // TPG)
    hrows = _ceil_to(N + 1, P)             # +1 virtual bias row
    vrow = N                               # virtual node index
    # gather windows: smallest count of equal windows covering hrows with
    # window size <= 32767
    nblk = max(1, -(-hrows // 32512))
    blkr = _ceil_to(-(-hrows // nblk), P)  # rows per window
    nblk = -(-hrows // blkr)
    assert blkr <= 32767

    pl = Plan()
    pl.N, pl.IN_F, pl.OUT_F = N, IN_F, OUT_F
    pl.ndst, pl.ndst_pad, pl.ntile, pl.ngrp = ndst, ndst_pad, ntile, ngrp
    pl.hrows, pl.vrow, pl.nblk, pl.blkr = hrows, vrow, nblk, blkr
    pl.kc = IN_F // P                      # K chunks for projection

    # --- dense inputs ---
    xT = np.zeros((pl.kc, P, hrows), np.float16)
    xT[:, :, :N] = np.ascontiguousarray(x.astype(np.float16).T).reshape(
        pl.kc, P, N
    )
    pl.xT = xT
    pl.W = np.ascontiguousarray(
        weight.astype(np.float16).reshape(pl.kc, P, OUT_F).transpose(1, 0, 2)
    )  # [P, kc, OUT_F]
    pl.bvec = np.ascontiguousarray(bias.astype(np.float16)[None, :])  # [1, OUT_F]

    # --- edges (+ virtual bias edges) ---
    deg_w = np.bincount(edge_dst, weights=edge_vals.astype(np.float64),
                        minlength=N).astype(np.float32)
    src_a = np.concatenate([edge_src.astype(np.int64),
                            np.full(N, vrow, np.int64)])
    dst_a = np.concatenate([edge_dst.astype(np.int64), np.arange(N)])
    val_a = np.concatenate([edge_vals.astype(np.float32), deg_w])

    core = dst_a // ndst
    dl = dst_a % ndst
    tile = dl // P
    grp = tile // TPG
    blk = src_a // blkr
    # stream order per core: (grp, blk, tile, dl)
    order = np.lexsort((dl, tile, blk, grp, core))
    src_a, dst_a, val_a = src_a[order], dst_a[order], val_a[order]
    core, dl, tile, grp, blk = (core[order], dl[order], tile[order],
                                grp[order], blk[order])

    # run = consecutive edges with same (core, tile, blk, dl)
    key_change = np.ones(len(src_a), bool)
    if len(src_a) > 1:
        key_change[1:] = ((core[1:] != core[:-1]) | (blk[1:] != blk[:-1]) |
                          (grp[1:] != grp[:-1]) | (tile[1:] != tile[:-1]) |
                          (dl[1:] != dl[:-1]))
    run_starts = np.nonzero(key_change)[0]
    run_lens = np.diff(np.append(run_starts, len(src_a)))
    r_core = core[run_starts]
    r_grp = grp[run_starts]
    r_blk = blk[run_starts]
    r_tile = tile[run_starts]
    r_dl = dl[run_starts]

    # --- greedy chunk/segment layout, per bucket (core, grp, blk, tile) ---
    # First pass: per-core chunk counts per bucket; second: final layout with
    # static (max-over-core) chunk counts.
    nbuck = ntile * nblk                   # buckets per core (tile-major id)

    def bucket_id(tile_, blk_):
        return tile_ * nblk + blk_

    # greedy chunking of one bucket given run lengths; returns pieces:
    # (run_index, take, chunk, slot) and chunk count
    def greedy(lens):
        pieces = []
        c, s, d = 0, 0, 0
        for ri, ln in enumerate(lens):
            rem = ln
            while rem > 0:
                if s == P or d == MSEG:
                    c += 1
                    s, d = 0, 0
                take = min(P - s, rem)
                pieces.append((ri, take, c, s, d))
                s += take
                d += 1
                rem -= take
        return pieces, (c + 1 if (s > 0 or c == 0) else c)

    # group runs per (core, bucket)
    rb = (r_core * nbuck + bucket_id(r_tile, r_blk)).astype(np.int64)
    rb_order = np.argsort(rb, kind="stable")
    # chunk counts per (core,bucket)
    chunks_cb = np.zeros((N_CORES, nbuck), np.int64)
    bucket_pieces = {}
    i = 0
    rb_sorted = rb[rb_order]
    while i < len(rb_sorted):
        j = i
        while j < len(rb_sorted) and rb_sorted[j] == rb_sorted[i]:
            j += 1
        ridx = rb_order[i:j]
        cb = int(rb_sorted[i])
        pieces, nch = greedy(run_lens[ridx])
        bucket_pieces[cb] = (ridx, pieces)
        chunks_cb[cb // nbuck, cb % nbuck] = nch
        i = j

    chunks_b = chunks_cb.max(axis=0)       # static per-bucket chunk count
    # pad each group's chunk count to a multiple of 16 (PSUM packing) by
    # growing the group's last bucket
    grp_tiles = [list(range(g * TPG, min((g + 1) * TPG, ntile)))
                 for g in range(ngrp)]
    for g in range(ngrp):
        bids = [bucket_id(t, b) for b in range(nblk) for t in grp_tiles[g]]
        tot = int(sum(chunks_b[b] for b in bids))
        pad = _ceil_to(max(tot, 16), 16) - tot
        chunks_b[bucket_id(grp_tiles[g][-1], nblk - 1)] += pad

    # static stream offsets (in chunks) bucket-by-bucket in processing order
    # processing order within group: blk-major, tile-minor
    chunk_off_b = np.zeros(nbuck, np.int64)   # chunk offset of bucket
    order_bids = []
    off = 0
    grp_chunk_off = []
    for g in range(ngrp):
        grp_chunk_off.append(off)
        for b in range(nblk):
            for t in grp_tiles[g]:
                bid = bucket_id(t, b)
                chunk_off_b[bid] = off
                order_bids.append(bid)
                off += int(chunks_b[bid])
    CC = off                                 # total chunks per core
    grp_chunk_off.append(CC)
    TOT = CC * P                             # total edge slots per core

    # gather calls: one per (grp, blk); sizes static
    gather_sizes = np.zeros((ngrp, nblk), np.int64)
    gather_off = np.zeros((ngrp, nblk), np.int64)
    for g in range(ngrp):
        for b in range(nblk):
            sz = sum(int(chunks_b[bucket_id(t, b)]) for t in grp_tiles[g])
            gather_sizes[g, b] = sz * P
    # offsets follow stream order
    acc = 0
    for g in range(ngrp):
        for b in range(nblk):
            gather_off[g, b] = acc
            acc += int(gather_sizes[g, b])
    assert acc == TOT

    # --- fill per-core slot arrays ---
    slot_src = np.zeros((N_CORES, TOT), np.int16)    # idx within window
    Bf = np.zeros((N_CORES, P, CC * MSEG), np.float16)
    # segment bookkeeping for S2
    seg_chunk, seg_slot, seg_dl, seg_core = [], [], [], []
    for cb, (ridx, pieces) in bucket_pieces.items():
        ci, bid = cb // nbuck, cb % nbuck
        base_c = int(chunk_off_b[bid])
        # vectorized expansion
        pr = np.array([p[0] for p in pieces])
        pt = np.array([p[1] for p in pieces])
        pc = np.array([p[2] for p in pieces]) + base_c
        ps_ = np.array([p[3] for p in pieces])
        pd = np.array([p[4] for p in pieces])
        gri = ridx[pr]
        # edge positions: run ri occupies run_starts[ri] .. +run_lens
        # pieces of a run are in order; compute per-piece source start
        # (offset within run): cumsum of takes per run
        src_off = np.zeros(len(pieces), np.int64)
        for k in range(1, len(pieces)):
            if pr[k] == pr[k - 1]:
                src_off[k] = src_off[k - 1] + pt[k - 1]
        e_start = run_starts[gri] + src_off            # into sorted edges
        slot_start = pc * P + ps_                      # into slot arrays
        # expand pieces to edges
        rep = np.repeat(np.arange(len(pieces)), pt)
        within = np.arange(len(rep)) - np.repeat(
            np.concatenate([[0], np.cumsum(pt)[:-1]]), pt)
        e_idx = e_start[rep] + within
        sl_idx = slot_start[rep] + within
        slot_src[ci, sl_idx] = (src_a[e_idx] - blk[e_idx] * blkr).astype(
            np.int16)
        Bf[ci, sl_idx % P, (sl_idx // P) * MSEG + pd[rep]] = (
            val_a[e_idx].astype(np.float16))
        seg_chunk.append(pc)
        seg_slot.append(pd)
        seg_dl.append(r_dl[gri])
        seg_core.append(np.full(len(pieces), ci))

    seg_chunk = np.concatenate(seg_chunk)
    seg_slot = np.concatenate(seg_slot)
    seg_dl = np.concatenate(seg_dl)
    seg_core = np.concatenate(seg_core)

    # --- L2 program (static): chunk -> tile is static via buckets ---
    chunk_tile = np.zeros(CC, np.int64)
    for bid in range(nbuck):
        t = bid // nblk
        c0 = chunk_off_b[bid]
        chunk_tile[c0:c0 + chunks_b[bid]] = t
    # L2 chunk j covers L1 chunks 4j..4j+3; may touch several tiles
    NL2 = CC // 4
    l2_mms = []                 # list of (j, tile) in program order
    for j in range(NL2):
        tiles_here = sorted(set(chunk_tile[4 * j:4 * j + 4].tolist()))
        for t in tiles_here:
            l2_mms.append((j, t))
    NMM = len(l2_mms)
    mm_index = {jt: i for i, jt in enumerate(l2_mms)}
    # start/stop flags per (group, tile)
    mm_start = np.zeros(NMM, bool)
    mm_stop = np.zeros(NMM, bool)
    seen = {}
    for i, (j, t) in enumerate(l2_mms):
        g = next(gg for gg in range(ngrp)
                 if grp_chunk_off[gg] <= 4 * j < grp_chunk_off[gg + 1])
        if (g, t) not in seen:
            mm_start[i] = True
        seen[(g, t)] = i
    for (g, t), i in seen.items():
        mm_stop[i] = True

    # S2 data
    S2f = np.zeros((N_CORES, P, NMM * P), np.float16)
    s_j = seg_chunk // 4
    s_row = (seg_chunk % 4) * MSEG + seg_slot
    s_tile = seg_dl // P
    s_mm = np.array([mm_index[(int(j), int(t))]
                     for j, t in zip(s_j, s_tile)])
    S2f[seg_core, s_row, s_mm * P + (seg_dl % P)] = np.float16(1.0)

    # idx tensor: per gather call, slot q -> [q % 16, off16 + q // 16],
    # replicated 8x across the 128 partitions (one copy per SDMA pair)
    IDX = np.zeros((N_CORES, 16, TOT // 16), np.int16)
    for g in range(ngrp):
        for b in range(nblk):
            o, n = int(gather_off[g, b]), int(gather_sizes[g, b])
            if n == 0:
                continue
            IDX[:, :, o // 16:(o + n) // 16] = slot_src[
                :, o:o + n].reshape(N_CORES, n // 16, 16).transpose(0, 2, 1)
    IDX = np.tile(IDX, (1, 8, 1))          # -> [N_CORES, 128, TOT // 16]

    pl.deg_w = deg_w
    pl.chunks_b, pl.chunk_off_b = chunks_b, chunk_off_b
    pl.grp_tiles, pl.grp_chunk_off = grp_tiles, grp_chunk_off
    pl.CC, pl.TOT, pl.NL2, pl.NMM = CC, TOT, NL2, NMM
    pl.gather_sizes, pl.gather_off = gather_sizes, gather_off
    pl.l2_mms, pl.mm_start, pl.mm_stop = l2_mms, mm_start, mm_stop
    pl.IDX, pl.Bf, pl.S2f = IDX, Bf, S2f
    return pl


# ---------------------------------------------------------------------------
# Device program
# ---------------------------------------------------------------------------

def build_bass(pl):
    import os
    STAGE = int(os.environ.get("K_STAGE", "4"))
    REP1 = int(os.environ.get("K_REP1", "1"))   # phase-1 repeats (timing)
    REP2 = int(os.environ.get("K_REP2", "1"))   # phase-2 repeats (timing)
    import concourse.bass as bass
    import concourse.mybir as mybir
    import concourse.tile as tile
    from concourse import bacc

    f16 = mybir.dt.float16
    f32 = mybir.dt.float32
    i16 = mybir.dt.int16

    NSWQ = int(os.environ.get("K_NSWQ", "4"))
    nc = bacc.Bacc("TRN2", target_bir_lowering=False, debug=False,
                   num_swdge_queues=NSWQ)

    xT_d = nc.dram_tensor("xt", [pl.kc, P, pl.hrows], f16,
                          kind="ExternalInput")
    W_d = nc.dram_tensor("w", [P, pl.kc, pl.OUT_F], f16, kind="ExternalInput")
    b_d = nc.dram_tensor("bvec", [1, pl.OUT_F], f16, kind="ExternalInput")
    idx_d = nc.dram_tensor("idx", [P, pl.TOT // 16], i16,
                           kind="ExternalInput")
    B_d = nc.dram_tensor("bmat", [P, pl.CC * MSEG], f16, kind="ExternalInput")
    S2_d = nc.dram_tensor("s2", [P, pl.NMM * P], f16, kind="ExternalInput")
    out_d = nc.dram_tensor("out", [pl.ndst_pad, pl.OUT_F], f32,
                           kind="ExternalOutput")
    h_d = nc.dram_tensor("hbuf", [pl.hrows, pl.OUT_F], f16)

    OF = pl.OUT_F
    RB = 512                       # projection row-batch
    n_rb = pl.hrows // RB if pl.hrows % RB == 0 else pl.hrows // RB + 1

    with tile.TileContext(nc) as tc:
        # ---------------- Phase 1: h = x @ W ----------------
        with (
            tc.tile_pool(name="pconst", bufs=1) as pconst,
            tc.tile_pool(name="pxt", bufs=3) as pxt,
            tc.tile_pool(name="phs", bufs=3) as phs,
            tc.tile_pool(name="ppsum", bufs=2, space="PSUM") as ppsum,
        ):
            W_sb = pconst.tile([P, pl.kc, OF], f16)
            nc.sync.dma_start(W_sb[:], W_d[:])
            b_sb = pconst.tile([1, OF], f16)
            nc.sync.dma_start(b_sb[:], b_d[:])

            for _rep1 in range(REP1):
                for j in range(n_rb):
                    r0 = j * RB
                    nrows = min(RB, pl.hrows - r0)
                    nch = nrows // P
                    xt = pxt.tile([P, pl.kc, RB], f16, tag="xt")
                    nc.sync.dma_start(
                        xt[:, :, :nrows],
                        xT_d[:, :, r0:r0 + nrows].rearrange("k p c -> p k c"),
                    )
                    ps = ppsum.tile([P, RB], f32, tag="pj")
                    for rc in range(nch):
                        for k in range(pl.kc):
                            nc.tensor.matmul(
                                ps[:, rc * P:(rc + 1) * P],
                                lhsT=xt[:, k, rc * P:(rc + 1) * P],
                                rhs=W_sb[:, k, :],
                                start=(k == 0),
                                stop=(k == pl.kc - 1),
                            )
                    hs = phs.tile([P, RB], f16, tag="hs")
                    nc.vector.tensor_copy(hs[:, :nrows], ps[:, :nrows])
                    nc.sync.dma_start(
                        h_d[r0:r0 + nrows, :].rearrange("(c p) f -> p c f", p=P),
                        hs[:, :nrows].rearrange("p (c f) -> p c f", f=OF),
                    )
            # virtual bias row - written after the projection loop so the
            # projection's batch covering this row cannot clobber it
            nc.sync.dma_start(h_d[pl.vrow:pl.vrow + 1, :], b_sb[:])

        # ---------------- Phase 2: gather + L1 + L2 ----------------
        with (
            tc.tile_pool(name="pidx", bufs=2) as pidx,
            tc.tile_pool(name="pmsg", bufs=2) as pmsg,
            tc.tile_pool(name="pB", bufs=3) as pB,
            tc.tile_pool(name="pS2", bufs=3) as pS2,
            tc.tile_pool(name="pP", bufs=3) as pP,
            tc.tile_pool(name="pout", bufs=2) as pout,
            tc.tile_pool(name="psL1", bufs=2, space="PSUM") as psL1,
            tc.tile_pool(name="psL2", bufs=TPG, space="PSUM") as psL2,
        ):
            for _rep2 in range(REP2):
                mm_i = 0          # global L2 mm counter
                for g in range(pl.ngrp):
                    tiles_g = pl.grp_tiles[g]
                    c_lo, c_hi = pl.grp_chunk_off[g], pl.grp_chunk_off[g + 1]
                    nch_g = c_hi - c_lo
                    # gathers for this group
                    msgs = {}
                    for b in range(pl.nblk):
                        if STAGE < 1:
                            continue
                        n = int(pl.gather_sizes[g, b])
                        if n == 0:
                            continue
                        o = int(pl.gather_off[g, b])
                        ixt = pidx.tile([P, n // 16], i16, tag="idx")
                        nc.sync.dma_start(ixt[:], idx_d[:, o // 16:(o + n) // 16])
                        if STAGE < 2:
                            continue
                        mt = pmsg.tile([P, n // P, OF], f16, tag="msg")
                        r0 = b * pl.blkr
                        r1 = min(r0 + pl.blkr, pl.hrows)
                        GMAX = int(os.environ.get("K_GMAX", "8192"))
                        for q0 in range(0, n, GMAX):
                            qn = min(GMAX, n - q0)
                            nc.gpsimd.dma_gather(
                                out_ap=mt[:, q0 // P:(q0 + qn) // P, :],
                                in_ap=h_d[r0:r1, :],
                                idxs_ap=ixt[:, q0 // 16:(q0 + qn) // 16],
                                num_idxs=qn,
                                num_idxs_reg=qn,
                                elem_size=OF,
                                single_packet=False,
                            )
                        msgs[b] = (mt, o)

                    l2ps = {}
                    for t in tiles_g:
                        l2ps[t] = psL2.tile([P, OF], f32, tag="l2",
                                            name=f"l2ps_{g}_{t}")

                    # B tiles per psum-group of 16 chunks
                    n_pg = nch_g // 16 if STAGE >= 3 else 0
                    for pg in range(n_pg):
                        c0 = c_lo + pg * 16
                        Bt = pB.tile([P, 16 * MSEG], f16, tag="B")
                        nc.sync.dma_start(
                            Bt[:], B_d[:, c0 * MSEG:(c0 + 16) * MSEG])
                        ps1 = psL1.tile([P, 16 * MSEG], f32, tag="ps1")
                        for cc in range(16):
                            c = c0 + cc
                            # locate the msgs tile holding this chunk
                            so = c * P
                            b = None
                            for bb in range(pl.nblk):
                                o = int(pl.gather_off[g, bb])
                                n = int(pl.gather_sizes[g, bb])
                                if o <= so < o + n:
                                    b = bb
                                    break
                            mt, o = msgs[b]
                            ci = (so - o) // P
                            cg = cc % 4
                            w = (cc // 4) % 4
                            nc.tensor.matmul(
                                ps1[32 * cg:32 * (cg + 1), w * P:(w + 1) * P],
                                lhsT=Bt[:, cc * MSEG:(cc + 1) * MSEG],
                                rhs=mt[:, ci, :],
                                start=True,
                                stop=True,
                                tile_position=(0, 32 * cg),
                            )
                        Pt = pP.tile([P, 4, OF], f16, tag="P")
                        nc.vector.tensor_copy(
                            Pt[:].rearrange("p a b -> p (a b)"), ps1[:])
                        # L2 mms for the 4 L2-chunks of this psum-group
                        j0 = (c0 // 4)
                        mms_here = []
                        while mm_i < pl.NMM and pl.l2_mms[mm_i][0] < j0 + 4:
                            mms_here.append(mm_i)
                            mm_i += 1
                        if mms_here and STAGE >= 4:
                            s2t = pS2.tile([P, len(mms_here) * P], f16, tag="s2",
                                           name=f"s2_{g}_{pg}")
                            nc.sync.dma_start(
                                s2t[:],
                                S2_d[:, mms_here[0] * P:
                                     (mms_here[-1] + 1) * P])
                            for q, mi in enumerate(mms_here):
                                j, t = pl.l2_mms[mi]
                                nc.tensor.matmul(
                                    l2ps[t][:],
                                    lhsT=s2t[:, q * P:(q + 1) * P],
                                    rhs=Pt[:, j % 4, :],
                                    start=bool(pl.mm_start[mi]),
                                    stop=bool(pl.mm_stop[mi]),
                                )
                    # evict group's dst tiles
                    for t in tiles_g:
                        if STAGE < 1:
                            continue
                        ot = pout.tile([P, OF], f32, tag="out")
                        if STAGE >= 4:
                            nc.vector.tensor_copy(ot[:], l2ps[t][:])
                        else:
                            nc.gpsimd.memset(ot[:], 0.0)
                        nc.sync.dma_start(out_d[t * P:(t + 1) * P, :], ot[:])

    nc.compile()
    return nc


# ---------------------------------------------------------------------------
# Entry point
# ---------------------------------------------------------------------------

def kernel(x, edge_src, edge_dst, edge_vals, weight, bias,
           _want_trace=False, _n_cores=None):
    x = np.asarray(x)
    edge_src = np.asarray(edge_src)
    edge_dst = np.asarray(edge_dst)
    edge_vals = np.asarray(edge_vals)
    weight = np.asarray(weight)
    bias = np.asarray(bias)

    pl = build_plan(x, edge_src, edge_dst, edge_vals, weight, bias)
    nc = build_bass(pl)

    from concourse.bass_utils import run_bass_kernel_spmd

    ncores = N_CORES if _n_cores is None else _n_cores
    in_maps = []
    for ci in range(ncores):
        in_maps.append({
            "xt": pl.xT,
            "w": pl.W,
            "bvec": pl.bvec,
            "idx": np.ascontiguousarray(pl.IDX[ci]),
            "bmat": np.ascontiguousarray(pl.Bf[ci]),
            "s2": np.ascontiguousarray(pl.S2f[ci]),
        })
    res = run_bass_kernel_spmd(nc, in_maps, core_ids=list(range(ncores)),
                               trace=_want_trace)
    outs = [res.results[ci]["out"][:pl.ndst, :] for ci in range(ncores)]
    if ncores < N_CORES:
        outs += [np.zeros((pl.ndst, pl.OUT_F), np.float32)] * (N_CORES - ncores)
    full = np.concatenate(outs, axis=0).astype(np.float32)
    if _want_trace:
        kernel._last_results = res
    return full



# revision 10
# speedup vs baseline: 2.3855x; 2.3855x over previous
"""GCNConv kernel for 8x Trainium2 NeuronCores (Bass/Tile).

Reference computation:
    h = x @ W + b                  # [N, 256] @ [256, 128] -> [N, 128]
    out[i] = sum_{e: dst[e]=i} val[e] * h[src[e]]

Strategy (per core; SPMD - one program, per-core data):
  - dst nodes sharded 12500/core (output rows).  Edges partitioned by dst.
  - Phase 1: every core computes the full h (fp16) into 4 per-window DRAM
    tensors via PE matmuls (host passes x transposed + fp16), so phase-2
    gathers of window w can start as soon as window w is projected.
  - Bias handled as a rank-1 matmul per dst tile: out_tile += deg_w (x) bias
    (deg_w = per-dst sum of incident edge vals, host-precomputed).
  - Phase 2: per-edge h rows pulled on-chip with dma_gather (int16 indices,
    4 source windows of <=25088 rows), SWDGE queues cycled.  Per 128-edge
    chunk, a host-built staircase matrix B [128e x 32seg] fp16 (carrying
    val) is the stationary matmul operand -> PSUM partial segment sums.  A
    second host-built one-hot S2 [128seg x 128dst] matmul accumulates
    segments into per-dst-tile PSUM, which is written out fp32.
  - Chunks are packed at (group, window) granularity (crossing dst-tile
    boundaries) to minimize gather padding.
All data-dependent structure is padded to the max across cores so the same
program serves all 8 cores.
"""

import os
import sys

for _p in ("/opt/trn_rl_repo",):
    if _p not in sys.path:
        sys.path.insert(0, _p)

import numpy as np

P = 128
MSEG = 32            # segment slots per 128-edge chunk
TPG = 4              # dst tiles (of 128 dst) per processing group
N_CORES = 8
RB = 512             # projection row-batch


def _ceil_to(a, m):
    return -(-a // m) * m


class Plan:
    """Static (core-invariant) program structure + per-core data arrays."""


def build_plan(x, edge_src, edge_dst, edge_vals, weight, bias):
    N, IN_F = x.shape
    OUT_F = weight.shape[1]
    assert N % N_CORES == 0
    ndst = N // N_CORES                    # dst nodes per core
    ndst_pad = _ceil_to(ndst, P)
    ntile = ndst_pad // P                  # dst tiles per core
    ngrp = -(-ntile // TPG)
    hrows = _ceil_to(N, P)
    blkr = 25088                           # window rows (mult of 512, <32767)
    nblk = -(-hrows // blkr)
    wrows = [min(blkr, hrows - w * blkr) for w in range(nblk)]

    pl = Plan()
    pl.N, pl.IN_F, pl.OUT_F = N, IN_F, OUT_F
    pl.ndst, pl.ndst_pad, pl.ntile, pl.ngrp = ndst, ndst_pad, ntile, ngrp
    pl.hrows, pl.nblk, pl.blkr, pl.wrows = hrows, nblk, blkr, wrows
    pl.kc = IN_F // P
    pl.grp_tiles = [list(range(g * TPG, min((g + 1) * TPG, ntile)))
                    for g in range(ngrp)]

    # --- dense inputs ---
    xT = np.zeros((pl.kc, P, hrows), np.float16)
    xT[:, :, :N] = np.ascontiguousarray(x.astype(np.float16).T).reshape(
        pl.kc, P, N)
    pl.xT = xT
    pl.W = np.ascontiguousarray(
        weight.astype(np.float16).reshape(pl.kc, P, OUT_F).transpose(1, 0, 2))
    pl.bvec = np.ascontiguousarray(bias.astype(np.float16)[None, :])

    deg_w = np.bincount(edge_dst, weights=edge_vals.astype(np.float64),
                        minlength=N).astype(np.float32)
    degw = np.zeros((N_CORES, 1, ndst_pad), np.float16)
    degw[:, 0, :ndst] = deg_w.reshape(N_CORES, ndst).astype(np.float16)
    pl.degw = degw

    # --- edges, sorted for (core, grp, blk) streams ---
    src = edge_src.astype(np.int64)
    dst = edge_dst.astype(np.int64)
    val = edge_vals.astype(np.float32)
    E = len(src)
    core = dst // ndst
    dl = dst % ndst
    grp = (dl // P) // TPG
    blk = src // blkr
    order = np.lexsort((src, dl, blk, grp, core))
    src, dst, val = src[order], dst[order], val[order]
    core, dl, grp, blk = core[order], dl[order], grp[order], blk[order]

    nbuck = ngrp * nblk
    bucket = (core * ngrp + grp) * nblk + blk
    bcount = np.bincount(bucket, minlength=N_CORES * nbuck)
    bstart = np.concatenate([[0], np.cumsum(bcount)[:-1]])
    slot_in_b = np.arange(E) - bstart[bucket]

    run_change = np.ones(E, bool)
    run_change[1:] = (bucket[1:] != bucket[:-1]) | (dl[1:] != dl[:-1])

    # fast path: chunks are fixed 128-slot windows of each bucket stream
    def compute_cd(slot_in_b):
        chunk_loc = slot_in_b // P
        piece_flag = run_change | (slot_in_b % P == 0)
        piece_id = np.cumsum(piece_flag) - 1
        cs = np.where(slot_in_b % P == 0, piece_id, -1)
        first = np.maximum.accumulate(cs)
        d = piece_id - first
        return chunk_loc, piece_flag, d

    chunk_loc, piece_flag, d = compute_cd(slot_in_b)

    if d.max() >= MSEG:
        # rare: some 128-slot window has >MSEG runs; redo those buckets with
        # a greedy that closes chunks early (slot padding inside the bucket)
        bad = np.unique(bucket[d >= MSEG])
        for bb in bad:
            lo, n = bstart[bb], bcount[bb]
            hi = lo + n
            rc = run_change[lo:hi]
            starts = np.nonzero(rc)[0]
            lens = np.diff(np.append(starts, n))
            pos = np.empty(n, np.int64)
            c, s, dd = 0, 0, 0
            for st, ln in zip(starts, lens):
                rem, off = ln, 0
                while rem > 0:
                    if s == P or dd == MSEG:
                        c += 1
                        s, dd = 0, 0
                    take = min(P - s, rem)
                    pos[st + off:st + off + take] = c * P + s + np.arange(take)
                    s += take
                    dd += 1
                    rem -= take
                    off += take
            slot_in_b[lo:hi] = pos
        chunk_loc, piece_flag, d = compute_cd(slot_in_b)
        assert d.max() < MSEG

    # per-bucket chunk counts -> static maxima
    cc_b = np.zeros(N_CORES * nbuck, np.int64)
    has = bcount > 0
    last_idx = bstart + bcount - 1
    cc_b[has] = chunk_loc[last_idx[has]] + 1
    cc_cb = cc_b.reshape(N_CORES, ngrp, nblk)
    CH = cc_cb.max(axis=0)                 # [ngrp, nblk] static chunk counts
    CC_g = CH.sum(axis=1)                  # per-group chunks
    CC = int(CC_g.sum())
    TOT = CC * P

    choff = np.zeros((ngrp, nblk), np.int64)
    c_lo = np.zeros(ngrp + 1, np.int64)
    off = 0
    for g in range(ngrp):
        c_lo[g] = off
        for b in range(nblk):
            choff[g, b] = off
            off += int(CH[g, b])
    c_lo[ngrp] = off
    assert off == CC

    chunk_glob = choff[grp, blk] + chunk_loc
    slot_glob = choff[grp, blk] * P + slot_in_b

    # idx array (int16 window-local row), wrapped per 16, replicated x8
    idx_flat = np.zeros((N_CORES, TOT), np.int16)
    idx_flat[core, slot_glob] = (src - blk * blkr).astype(np.int16)
    IDX = np.ascontiguousarray(
        idx_flat.reshape(N_CORES, TOT // 16, 16).transpose(0, 2, 1))
    IDX = np.tile(IDX, (1, 8, 1))          # [N_CORES, 128, TOT // 16]
    pl.IDX = IDX

    # B staircase (vals)
    Bf = np.zeros((N_CORES, P, CC * MSEG), np.float16)
    Bf[core, slot_glob % P, chunk_glob * MSEG + d] = val.astype(np.float16)
    pl.Bf = Bf

    # segments (pieces) -> L2 one-hot S2
    pidx = np.nonzero(piece_flag)[0]
    p_core = core[pidx]
    p_g = grp[pidx]
    p_crel = chunk_glob[pidx] - c_lo[p_g]
    p_j = p_crel // 4
    p_band = p_crel % 4
    p_d = d[pidx]
    p_dl = dl[pidx]
    p_tile = p_dl // P                     # 0..ntile-1

    # program-static mm list: union over cores of (g, j, tile)
    key = (p_g * 100000 + p_j) * 1000 + p_tile
    ukey = np.unique(key)
    mm_g = ukey // 100000000
    mm_j = (ukey // 1000) % 100000
    mm_t = ukey % 1000
    NMM = len(ukey)
    mm_of_key = {int(k): i for i, k in enumerate(ukey)}
    p_mm = np.searchsorted(ukey, key)

    S2f = np.zeros((N_CORES, P, NMM * P), np.float16)
    S2f[p_core, p_band * MSEG + p_d, p_mm * P + (p_dl % P)] = np.float16(1.0)
    pl.S2f = S2f

    # per-(g,tile) mm bookkeeping for start/stop flags
    mm_stop = np.zeros(NMM, bool)
    seen = {}
    for i in range(NMM):
        seen[(int(mm_g[i]), int(mm_t[i]))] = i
    for (_, _), i in seen.items():
        mm_stop[i] = True
    # first mm index per group (columns of S2 are mm-ordered = (g, j, t))
    mm0_g = np.searchsorted(mm_g, np.arange(ngrp), side="left")
    nmm_g = np.searchsorted(mm_g, np.arange(ngrp), side="right") - mm0_g

    pl.CC, pl.TOT, pl.NMM = CC, TOT, NMM
    pl.CH, pl.CC_g, pl.choff, pl.c_lo = CH, CC_g, choff, c_lo
    pl.mm_g, pl.mm_j, pl.mm_t, pl.mm_stop = mm_g, mm_j, mm_t, mm_stop
    pl.mm0_g, pl.nmm_g = mm0_g, nmm_g
    return pl


# ---------------------------------------------------------------------------
# Device program
# ---------------------------------------------------------------------------

def build_bass(pl):
    import concourse.bass as bass
    import concourse.mybir as mybir
    import concourse.tile as tile
    from concourse import bacc

    f16 = mybir.dt.float16
    f32 = mybir.dt.float32
    i16 = mybir.dt.int16

    NSWQ = int(os.environ.get("K_NSWQ", "4"))
    GMAX = int(os.environ.get("K_GMAX", "8192"))
    nc = bacc.Bacc("TRN2", target_bir_lowering=False, debug=False,
                   num_swdge_queues=NSWQ)

    OF = pl.OUT_F
    xT_d = nc.dram_tensor("xt", [pl.kc, P, pl.hrows], f16,
                          kind="ExternalInput")
    W_d = nc.dram_tensor("w", [P, pl.kc, OF], f16, kind="ExternalInput")
    b_d = nc.dram_tensor("bvec", [1, OF], f16, kind="ExternalInput")
    dw_d = nc.dram_tensor("degw", [1, pl.ndst_pad], f16,
                          kind="ExternalInput")
    idx_d = nc.dram_tensor("idx", [P, pl.TOT // 16], i16,
                           kind="ExternalInput")
    B_d = nc.dram_tensor("bmat", [P, pl.CC * MSEG], f16,
                         kind="ExternalInput")
    S2_d = nc.dram_tensor("s2", [P, pl.NMM * P], f16, kind="ExternalInput")
    out_d = nc.dram_tensor("out", [pl.ndst_pad, OF], f32,
                           kind="ExternalOutput")
    h_ds = [nc.dram_tensor(f"hbuf{w}", [pl.wrows[w], OF], f16)
            for w in range(pl.nblk)]

    qctr = [0]

    def next_q():
        q = qctr[0] % NSWQ
        qctr[0] += 1
        return q

    with tile.TileContext(nc) as tc:
        with tc.tile_pool(name="pconst", bufs=1) as pconst:
            W_sb = pconst.tile([P, pl.kc, OF], f16)
            nc.sync.dma_start(W_sb[:], W_d[:])
            # deg_w / bias padded to K=128 (row 0 live, rest zero) so the
            # rank-1 bias matmul uses the same PE tile config as the S2 mms
            dw2 = pconst.tile([P, pl.ndst_pad], f16)
            nc.gpsimd.memset(dw2[:], 0.0)
            nc.sync.dma_start(dw2[0:1, :], dw_d[:])
            b2 = pconst.tile([P, OF], f16)
            nc.gpsimd.memset(b2[:], 0.0)
            nc.sync.dma_start(b2[0:1, :], b_d[:])

            # ------------- Phase 1: h = x @ W (per window) -------------
            with (
                tc.tile_pool(name="pxt", bufs=3) as pxt,
                tc.tile_pool(name="phs", bufs=3) as phs,
                tc.tile_pool(name="ppsum", bufs=2, space="PSUM") as ppsum,
            ):
                for w in range(pl.nblk):
                    r0g = w * pl.blkr
                    for rb in range(-(-pl.wrows[w] // RB)):
                        r0 = rb * RB
                        nrows = min(RB, pl.wrows[w] - r0)
                        nch = nrows // P
                        xt = pxt.tile([P, pl.kc, RB], f16, tag="xt")
                        nc.sync.dma_start(
                            xt[:, :, :nrows],
                            xT_d[:, :, r0g + r0:r0g + r0 + nrows].rearrange(
                                "k p c -> p k c"),
                        )
                        ps = ppsum.tile([P, RB], f32, tag="pj")
                        for rc in range(nch):
                            for k in range(pl.kc):
                                nc.tensor.matmul(
                                    ps[:, rc * P:(rc + 1) * P],
                                    lhsT=xt[:, k, rc * P:(rc + 1) * P],
                                    rhs=W_sb[:, k, :],
                                    start=(k == 0),
                                    stop=(k == pl.kc - 1),
                                )
                        hs = phs.tile([P, RB], f16, tag="hs")
                        nc.vector.tensor_copy(hs[:, :nrows], ps[:, :nrows])
                        nc.sync.dma_start(
                            h_ds[w][r0:r0 + nrows, :].rearrange(
                                "(c p) f -> p c f", p=P),
                            hs[:, :nrows].rearrange("p (c f) -> p c f", f=OF),
                        )

            # ---------------- Phase 2: gather + L1 + L2 ----------------
            with (
                tc.tile_pool(name="pidx", bufs=2) as pidx,
                tc.tile_pool(name="pmsg", bufs=3) as pmsg,
                tc.tile_pool(name="pB", bufs=2) as pB,
                tc.tile_pool(name="pS2", bufs=2) as pS2,
                tc.tile_pool(name="pP",
                             bufs=int(max(-(-pl.CC_g // 16))) + 2) as pP,
                tc.tile_pool(name="pout", bufs=2) as pout,
                tc.tile_pool(name="psL1", bufs=2, space="PSUM") as psL1,
                tc.tile_pool(name="psL2", bufs=2, space="PSUM") as psL2,
            ):
                for g in range(pl.ngrp):
                    tiles_g = pl.grp_tiles[g]
                    ccg = int(pl.CC_g[g])
                    clo = int(pl.c_lo[g])
                    # group idx tile + gathers per window
                    ixg = pidx.tile([P, ccg * 8], i16, tag="idx")
                    nc.sync.dma_start(
                        ixg[:], idx_d[:, clo * 8:(clo + ccg) * 8])
                    msgs = {}
                    for b in range(pl.nblk):
                        n = int(pl.CH[g, b]) * P
                        if n == 0:
                            continue
                        o = int(pl.choff[g, b]) * P     # global slot offset
                        ol = o - clo * P                # offset within group
                        mt = pmsg.tile([P, n // P, OF], f16, tag="msg")
                        for q0 in range(0, n, GMAX):
                            qn = min(GMAX, n - q0)
                            nc.gpsimd.dma_gather(
                                out_ap=mt[:, q0 // P:(q0 + qn) // P, :],
                                in_ap=h_ds[b][:, :],
                                idxs_ap=ixg[:, (ol + q0) // 16:
                                            (ol + q0 + qn) // 16],
                                num_idxs=qn,
                                num_idxs_reg=qn,
                                elem_size=OF,
                                single_packet=False,
                                queue_num=next_q(),
                            )
                        msgs[b] = mt

                    # B + S2 for the whole group
                    Bt = pB.tile([P, ccg * MSEG], f16, tag="B")
                    nc.sync.dma_start(
                        Bt[:], B_d[:, clo * MSEG:(clo + ccg) * MSEG])
                    mm0 = int(pl.mm0_g[g])
                    nmm = int(pl.nmm_g[g])
                    s2t = pS2.tile([P, max(nmm, 1) * P], f16, tag="s2")
                    if nmm:
                        nc.sync.dma_start(
                            s2t[:, :nmm * P],
                            S2_d[:, mm0 * P:(mm0 + nmm) * P])

                    # window of each chunk (static)
                    win_of = np.repeat(np.arange(pl.nblk),
                                       pl.CH[g]).astype(np.int64)

                    pts = []
                    n_pg = -(-ccg // 16)
                    for pg in range(n_pg):
                        cc0 = 16 * pg
                        nch = min(16, ccg - cc0)
                        nw2 = -(-nch // 4)
                        ps1 = psL1.tile([P, 16 * MSEG], f32, tag="ps1")
                        for cc2 in range(nch):
                            crel = cc0 + cc2
                            b = int(win_of[crel])
                            ci = clo + crel - int(pl.choff[g, b])
                            band = cc2 % 4
                            w2 = cc2 // 4
                            nc.tensor.matmul(
                                ps1[MSEG * band:MSEG * (band + 1),
                                    w2 * P:(w2 + 1) * P],
                                lhsT=Bt[:, crel * MSEG:(crel + 1) * MSEG],
                                rhs=msgs[b][:, ci, :],
                                start=True,
                                stop=True,
                                tile_position=(0, MSEG * band),
                            )
                        Pt = pP.tile([P, 4, OF], f16, tag="P")
                        nc.vector.tensor_copy(
                            Pt[:, :nw2, :].rearrange("p a b -> p (a b)"),
                            ps1[:, :nw2 * P])
                        pts.append(Pt)

                    # L2: one CONTIGUOUS matmul burst per dst tile (PSUM
                    # accumulation groups must not interleave within a bank)
                    # all TPG dst-tile accumulators packed into one bank
                    l2t = psL2.tile([P, TPG, OF], f32, tag="l2",
                                    name=f"l2ps_{g}")
                    for t in tiles_g:
                        my_mms = [i for i in range(mm0, mm0 + nmm)
                                  if int(pl.mm_t[i]) == t]
                        reg = l2t[:, t - g * TPG, :]
                        # bias: out_tile += deg_w (x) bias   (rank-1)
                        nc.tensor.matmul(
                            reg,
                            lhsT=dw2[:, t * P:(t + 1) * P],
                            rhs=b2[:],
                            start=True,
                            stop=not my_mms,
                        )
                        for q, i in enumerate(my_mms):
                            j = int(pl.mm_j[i])
                            nc.tensor.matmul(
                                reg,
                                lhsT=s2t[:, (i - mm0) * P:(i - mm0 + 1) * P],
                                rhs=pts[j // 4][:, j % 4, :],
                                start=False,
                                stop=(q == len(my_mms) - 1),
                            )

                    ntg = len(tiles_g)
                    t0 = tiles_g[0]
                    ot = pout.tile([P, TPG, OF], f32, tag="out")
                    nc.vector.tensor_copy(
                        ot[:, :ntg, :].rearrange("p a b -> p (a b)"),
                        l2t[:, :ntg, :].rearrange("p a b -> p (a b)"))
                    nc.sync.dma_start(
                        out_d[t0 * P:(t0 + ntg) * P, :].rearrange(
                            "(t p) f -> p t f", p=P),
                        ot[:, :ntg, :])

    nc.compile()
    return nc


# ---------------------------------------------------------------------------
# Entry point
# ---------------------------------------------------------------------------

def kernel(x, edge_src, edge_dst, edge_vals, weight, bias,
           _want_trace=False, _n_cores=None):
    x = np.asarray(x)
    edge_src = np.asarray(edge_src)
    edge_dst = np.asarray(edge_dst)
    edge_vals = np.asarray(edge_vals)
    weight = np.asarray(weight)
    bias = np.asarray(bias)

    pl = build_plan(x, edge_src, edge_dst, edge_vals, weight, bias)
    nc = build_bass(pl)

    from concourse.bass_utils import run_bass_kernel_spmd

    ncores = N_CORES if _n_cores is None else _n_cores
    in_maps = []
    for ci in range(ncores):
        in_maps.append({
            "xt": pl.xT,
            "w": pl.W,
            "bvec": pl.bvec,
            "degw": np.ascontiguousarray(pl.degw[ci]),
            "idx": np.ascontiguousarray(pl.IDX[ci]),
            "bmat": np.ascontiguousarray(pl.Bf[ci]),
            "s2": np.ascontiguousarray(pl.S2f[ci]),
        })
    res = run_bass_kernel_spmd(nc, in_maps, core_ids=list(range(ncores)),
                               trace=_want_trace)
    outs = [res.results[ci]["out"][:pl.ndst, :] for ci in range(ncores)]
    if ncores < N_CORES:
        outs += [np.zeros((pl.ndst, pl.OUT_F), np.float32)] * (
            N_CORES - ncores)
    full = np.concatenate(outs, axis=0).astype(np.float32)
    if _want_trace:
        kernel._last_results = res
    return full


# revision 13
# speedup vs baseline: 3.4636x; 1.4519x over previous
"""GCNConv kernel for 8x Trainium2 NeuronCores (Bass/Tile).

Reference computation:
    h = x @ W + b                  # [N, 256] @ [256, 128] -> [N, 128]
    out[i] = sum_{e: dst[e]=i} val[e] * h[src[e]]

Strategy (per core; SPMD - one program, per-core data):
  - dst nodes sharded 12500/core (output rows).  Edges partitioned by dst.
  - Phase 1: every core computes the full h (fp16) into 4 per-window DRAM
    tensors via PE matmuls (host passes x transposed + fp16), so phase-2
    gathers of window w can start as soon as window w is projected.
  - Bias handled as a rank-1 matmul per dst tile: out_tile += deg_w (x) bias
    (deg_w = per-dst sum of incident edge vals, host-precomputed).
  - Phase 2: per-edge h rows pulled on-chip with dma_gather (int16 indices,
    4 source windows of <=25088 rows), SWDGE queues cycled.  Per 128-edge
    chunk, a host-built staircase matrix B [128e x 32seg] fp16 (carrying
    val) is the stationary matmul operand -> PSUM partial segment sums.  A
    second host-built one-hot S2 [128seg x 128dst] matmul accumulates
    segments into per-dst-tile PSUM, which is written out fp32.
  - Chunks are packed at (group, window) granularity (crossing dst-tile
    boundaries) to minimize gather padding.
All data-dependent structure is padded to the max across cores so the same
program serves all 8 cores.
"""

import os
import sys

for _p in ("/opt/trn_rl_repo",):
    if _p not in sys.path:
        sys.path.insert(0, _p)

import numpy as np

P = 128
MSEG = 32            # segment slots per 128-edge chunk
TPG = 4              # dst tiles (of 128 dst) per processing group
N_CORES = 8
RB = 512             # projection row-batch


def _ceil_to(a, m):
    return -(-a // m) * m


class Plan:
    """Static (core-invariant) program structure + per-core data arrays."""


def build_plan(x, edge_src, edge_dst, edge_vals, weight, bias):
    N, IN_F = x.shape
    OUT_F = weight.shape[1]
    assert N % N_CORES == 0
    ndst = N // N_CORES                    # dst nodes per core
    ndst_pad = _ceil_to(ndst, P)
    ntile = ndst_pad // P                  # dst tiles per core
    ngrp = -(-ntile // TPG)
    hrows = _ceil_to(N, P)
    blkr = 25088                           # window rows (mult of 512, <32767)
    nblk = -(-hrows // blkr)
    wrows = [min(blkr, hrows - w * blkr) for w in range(nblk)]

    pl = Plan()
    pl.N, pl.IN_F, pl.OUT_F = N, IN_F, OUT_F
    pl.ndst, pl.ndst_pad, pl.ntile, pl.ngrp = ndst, ndst_pad, ntile, ngrp
    pl.hrows, pl.nblk, pl.blkr, pl.wrows = hrows, nblk, blkr, wrows
    pl.kc = IN_F // P
    pl.grp_tiles = [list(range(g * TPG, min((g + 1) * TPG, ntile)))
                    for g in range(ngrp)]

    # --- dense inputs ---
    import ml_dtypes
    f8 = ml_dtypes.float8_e4m3
    xf8 = os.environ.get("K_XF8", "0") == "1"
    xdt = f8 if xf8 else np.float16
    pl.xf8 = xf8
    xT = np.zeros((pl.kc, P, hrows), xdt)
    xT[:, :, :N] = np.ascontiguousarray(x.astype(np.float32).T).reshape(
        pl.kc, P, N).astype(xdt)
    pl.xT = xT
    pl.W = np.ascontiguousarray(
        weight.astype(np.float32).reshape(pl.kc, P, OUT_F).transpose(
            1, 0, 2)).astype(xdt)
    pl.bvec = np.ascontiguousarray(bias.astype(np.float16)[None, :])

    deg_w = np.bincount(edge_dst, weights=edge_vals.astype(np.float64),
                        minlength=N).astype(np.float32)
    degw = np.zeros((N_CORES, 1, ndst_pad), np.float16)
    degw[:, 0, :ndst] = deg_w.reshape(N_CORES, ndst).astype(np.float16)
    pl.degw = degw

    # --- edges, sorted for (core, grp, blk) streams ---
    src = edge_src.astype(np.int64)
    dst = edge_dst.astype(np.int64)
    val = edge_vals.astype(np.float32)
    E = len(src)
    core = dst // ndst
    dl = dst % ndst
    grp = (dl // P) // TPG
    blk = src // blkr
    order = np.lexsort((src, dl, blk, grp, core))
    src, dst, val = src[order], dst[order], val[order]
    core, dl, grp, blk = core[order], dl[order], grp[order], blk[order]

    nbuck = ngrp * nblk
    bucket = (core * ngrp + grp) * nblk + blk
    bcount = np.bincount(bucket, minlength=N_CORES * nbuck)
    bstart = np.concatenate([[0], np.cumsum(bcount)[:-1]])
    slot_in_b = np.arange(E) - bstart[bucket]

    run_change = np.ones(E, bool)
    run_change[1:] = (bucket[1:] != bucket[:-1]) | (dl[1:] != dl[:-1])

    # fast path: chunks are fixed 128-slot windows of each bucket stream
    def compute_cd(slot_in_b):
        chunk_loc = slot_in_b // P
        piece_flag = run_change | (slot_in_b % P == 0)
        piece_id = np.cumsum(piece_flag) - 1
        cs = np.where(slot_in_b % P == 0, piece_id, -1)
        first = np.maximum.accumulate(cs)
        d = piece_id - first
        return chunk_loc, piece_flag, d

    chunk_loc, piece_flag, d = compute_cd(slot_in_b)

    if d.max() >= MSEG:
        # rare: some 128-slot window has >MSEG runs; redo those buckets with
        # a greedy that closes chunks early (slot padding inside the bucket)
        bad = np.unique(bucket[d >= MSEG])
        for bb in bad:
            lo, n = bstart[bb], bcount[bb]
            hi = lo + n
            rc = run_change[lo:hi]
            starts = np.nonzero(rc)[0]
            lens = np.diff(np.append(starts, n))
            pos = np.empty(n, np.int64)
            c, s, dd = 0, 0, 0
            for st, ln in zip(starts, lens):
                rem, off = ln, 0
                while rem > 0:
                    if s == P or dd == MSEG:
                        c += 1
                        s, dd = 0, 0
                    take = min(P - s, rem)
                    pos[st + off:st + off + take] = c * P + s + np.arange(take)
                    s += take
                    dd += 1
                    rem -= take
                    off += take
            slot_in_b[lo:hi] = pos
        chunk_loc, piece_flag, d = compute_cd(slot_in_b)
        assert d.max() < MSEG

    # per-bucket chunk counts -> static maxima
    cc_b = np.zeros(N_CORES * nbuck, np.int64)
    has = bcount > 0
    last_idx = bstart + bcount - 1
    cc_b[has] = chunk_loc[last_idx[has]] + 1
    cc_cb = cc_b.reshape(N_CORES, ngrp, nblk)
    CH = cc_cb.max(axis=0)                 # [ngrp, nblk] static chunk counts
    CC_g = CH.sum(axis=1)                  # per-group chunks
    CC = int(CC_g.sum())
    TOT = CC * P

    choff = np.zeros((ngrp, nblk), np.int64)
    c_lo = np.zeros(ngrp + 1, np.int64)
    off = 0
    for g in range(ngrp):
        c_lo[g] = off
        for b in range(nblk):
            choff[g, b] = off
            off += int(CH[g, b])
    c_lo[ngrp] = off
    assert off == CC

    chunk_glob = choff[grp, blk] + chunk_loc
    slot_glob = choff[grp, blk] * P + slot_in_b

    # idx array (int16 window-local row), wrapped per 16, replicated x8
    idx_flat = np.zeros((N_CORES, TOT), np.int16)
    idx_flat[core, slot_glob] = (src - blk * blkr).astype(np.int16)
    IDX = np.ascontiguousarray(
        idx_flat.reshape(N_CORES, TOT // 16, 16).transpose(0, 2, 1))
    IDX = np.tile(IDX, (1, 8, 1))          # [N_CORES, 128, TOT // 16]
    pl.IDX = IDX

    # B staircase (vals)
    Bf = np.zeros((N_CORES, P, CC * MSEG), np.float16)
    Bf[core, slot_glob % P, chunk_glob * MSEG + d] = val.astype(np.float16)
    pl.Bf = Bf

    # segments (pieces) -> L2 one-hot S2
    pidx = np.nonzero(piece_flag)[0]
    p_core = core[pidx]
    p_g = grp[pidx]
    p_crel = chunk_glob[pidx] - c_lo[p_g]
    p_j = p_crel // 4
    p_band = p_crel % 4
    p_d = d[pidx]
    p_dl = dl[pidx]
    p_tile = p_dl // P                     # 0..ntile-1

    # program-static mm list: union over cores of (g, j, tile)
    key = (p_g * 100000 + p_j) * 1000 + p_tile
    ukey = np.unique(key)
    mm_g = ukey // 100000000
    mm_j = (ukey // 1000) % 100000
    mm_t = ukey % 1000
    NMM = len(ukey)
    mm_of_key = {int(k): i for i, k in enumerate(ukey)}
    p_mm = np.searchsorted(ukey, key)

    s2f8 = os.environ.get("K_S2F8", "0") == "1"
    S2f = np.zeros((N_CORES, P, NMM * P),
                   f8 if s2f8 else np.float16)
    S2f[p_core, p_band * MSEG + p_d, p_mm * P + (p_dl % P)] = 1.0
    pl.S2f = S2f
    pl.s2f8 = s2f8

    # per-(g,tile) mm bookkeeping for start/stop flags
    mm_stop = np.zeros(NMM, bool)
    seen = {}
    for i in range(NMM):
        seen[(int(mm_g[i]), int(mm_t[i]))] = i
    for (_, _), i in seen.items():
        mm_stop[i] = True
    # first mm index per group (columns of S2 are mm-ordered = (g, j, t))
    mm0_g = np.searchsorted(mm_g, np.arange(ngrp), side="left")
    nmm_g = np.searchsorted(mm_g, np.arange(ngrp), side="right") - mm0_g

    pl.CC, pl.TOT, pl.NMM = CC, TOT, NMM
    pl.CH, pl.CC_g, pl.choff, pl.c_lo = CH, CC_g, choff, c_lo
    pl.mm_g, pl.mm_j, pl.mm_t, pl.mm_stop = mm_g, mm_j, mm_t, mm_stop
    pl.mm0_g, pl.nmm_g = mm0_g, nmm_g
    return pl


# ---------------------------------------------------------------------------
# Device program
# ---------------------------------------------------------------------------

def build_bass(pl):
    import concourse.bass as bass
    import concourse.mybir as mybir
    import concourse.tile as tile
    from concourse import bacc

    f16 = mybir.dt.float16
    f32 = mybir.dt.float32
    f8 = mybir.dt.float8e4
    i16 = mybir.dt.int16

    NSWQ = int(os.environ.get("K_NSWQ", "4"))
    GMAX = int(os.environ.get("K_GMAX", "8192"))
    nc = bacc.Bacc("TRN2", target_bir_lowering=False, debug=False,
                   num_swdge_queues=NSWQ)

    OF = pl.OUT_F
    xdt = f8 if pl.xf8 else f16
    xT_d = nc.dram_tensor("xt", [pl.kc, P, pl.hrows], xdt,
                          kind="ExternalInput")
    W_d = nc.dram_tensor("w", [P, pl.kc, OF], xdt, kind="ExternalInput")
    b_d = nc.dram_tensor("bvec", [1, OF], f16, kind="ExternalInput")
    dw_d = nc.dram_tensor("degw", [1, pl.ndst_pad], f16,
                          kind="ExternalInput")
    idx_d = nc.dram_tensor("idx", [P, pl.TOT // 16], i16,
                           kind="ExternalInput")
    B_d = nc.dram_tensor("bmat", [P, pl.CC * MSEG], f16,
                         kind="ExternalInput")
    S2_d = nc.dram_tensor("s2", [P, pl.NMM * P],
                          f8 if pl.s2f8 else f16,
                          kind="ExternalInput")
    out_d = nc.dram_tensor("out", [pl.ndst_pad, OF], f32,
                           kind="ExternalOutput")
    h_ds = [nc.dram_tensor(f"hbuf{w}", [pl.wrows[w], OF], f16)
            for w in range(pl.nblk)]

    qctr = [0]

    def next_q():
        q = qctr[0] % NSWQ
        qctr[0] += 1
        return q

    with tile.TileContext(nc) as tc:
        with tc.tile_pool(name="pconst", bufs=1) as pconst:
            W_sb = pconst.tile([P, pl.kc, OF], xdt)
            nc.sync.dma_start(W_sb[:], W_d[:])
            # deg_w / bias padded to K=128 (row 0 live, rest zero) so the
            # rank-1 bias matmul uses the same PE tile config as the S2 mms
            dw2 = pconst.tile([P, pl.ndst_pad], f16)
            nc.gpsimd.memset(dw2[:], 0.0)
            nc.sync.dma_start(dw2[0:1, :], dw_d[:])
            b2 = pconst.tile([P, OF], f16)
            nc.gpsimd.memset(b2[:], 0.0)
            nc.sync.dma_start(b2[0:1, :], b_d[:])

            # ------------- Phase 1: h = x @ W (per window) -------------
            with (
                tc.tile_pool(name="pxt", bufs=3) as pxt,
                tc.tile_pool(name="phs", bufs=3) as phs,
                tc.tile_pool(name="ppsum", bufs=2, space="PSUM") as ppsum,
            ):
                for w in range(pl.nblk):
                    r0g = w * pl.blkr
                    for rb in range(-(-pl.wrows[w] // RB)):
                        r0 = rb * RB
                        nrows = min(RB, pl.wrows[w] - r0)
                        nch = nrows // P
                        xt = pxt.tile([P, pl.kc, RB], xdt, tag="xt")
                        nc.sync.dma_start(
                            xt[:, :, :nrows],
                            xT_d[:, :, r0g + r0:r0g + r0 + nrows].rearrange(
                                "k p c -> p k c"),
                        )
                        ps = ppsum.tile([P, RB], f32, tag="pj")
                        for rc in range(nch):
                            for k in range(pl.kc):
                                nc.tensor.matmul(
                                    ps[:, rc * P:(rc + 1) * P],
                                    lhsT=xt[:, k, rc * P:(rc + 1) * P],
                                    rhs=W_sb[:, k, :],
                                    start=(k == 0),
                                    stop=(k == pl.kc - 1),
                                )
                        hs = phs.tile([P, RB], f16, tag="hs")
                        nc.vector.tensor_copy(hs[:, :nrows], ps[:, :nrows])
                        nc.sync.dma_start(
                            h_ds[w][r0:r0 + nrows, :].rearrange(
                                "(c p) f -> p c f", p=P),
                            hs[:, :nrows].rearrange("p (c f) -> p c f", f=OF),
                        )

            # ---------------- Phase 2: gather + L1 + L2 ----------------
            with (
                tc.tile_pool(name="pidx", bufs=2) as pidx,
                tc.tile_pool(name="pmsg", bufs=6) as pmsg,
                tc.tile_pool(name="pB", bufs=2) as pB,
                tc.tile_pool(name="pS2", bufs=2) as pS2,
                tc.tile_pool(name="pP",
                             bufs=int(max(-(-pl.CC_g // 16))) + 2) as pP,
                tc.tile_pool(name="pout", bufs=2) as pout,
                tc.tile_pool(name="psL1", bufs=2, space="PSUM") as psL1,
                tc.tile_pool(name="psL2", bufs=2, space="PSUM") as psL2,
            ):
                for g in range(pl.ngrp):
                    tiles_g = pl.grp_tiles[g]
                    ccg = int(pl.CC_g[g])
                    clo = int(pl.c_lo[g])
                    # group idx tile + gathers per window
                    ixg = pidx.tile([P, ccg * 8], i16, tag="idx")
                    nc.sync.dma_start(
                        ixg[:], idx_d[:, clo * 8:(clo + ccg) * 8])
                    msgs = {}
                    for b in range(pl.nblk):
                        n = int(pl.CH[g, b]) * P
                        if n == 0:
                            continue
                        o = int(pl.choff[g, b]) * P     # global slot offset
                        ol = o - clo * P                # offset within group
                        mt = pmsg.tile([P, n // P, OF], f16, tag="msg")
                        for q0 in range(0, n, GMAX):
                            qn = min(GMAX, n - q0)
                            nc.gpsimd.dma_gather(
                                out_ap=mt[:, q0 // P:(q0 + qn) // P, :],
                                in_ap=h_ds[b][:, :],
                                idxs_ap=ixg[:, (ol + q0) // 16:
                                            (ol + q0 + qn) // 16],
                                num_idxs=qn,
                                num_idxs_reg=qn,
                                elem_size=OF,
                                single_packet=False,
                                queue_num=next_q(),
                            )
                        msgs[b] = mt

                    # B + S2 for the whole group
                    Bt = pB.tile([P, ccg * MSEG], f16, tag="B")
                    nc.sync.dma_start(
                        Bt[:], B_d[:, clo * MSEG:(clo + ccg) * MSEG])
                    mm0 = int(pl.mm0_g[g])
                    nmm = int(pl.nmm_g[g])
                    s2t = pS2.tile([P, max(nmm, 1) * P],
                   f8 if pl.s2f8 else f16, tag="s2")
                    if nmm:
                        nc.sync.dma_start(
                            s2t[:, :nmm * P],
                            S2_d[:, mm0 * P:(mm0 + nmm) * P])

                    # window of each chunk (static)
                    win_of = np.repeat(np.arange(pl.nblk),
                                       pl.CH[g]).astype(np.int64)

                    pts = []
                    n_pg = -(-ccg // 16)
                    for pg in range(n_pg):
                        cc0 = 16 * pg
                        nch = min(16, ccg - cc0)
                        nw2 = -(-nch // 4)
                        ps1 = psL1.tile([P, 16 * MSEG], f32, tag="ps1")
                        for cc2 in range(nch):
                            crel = cc0 + cc2
                            b = int(win_of[crel])
                            ci = clo + crel - int(pl.choff[g, b])
                            band = cc2 % 4
                            w2 = cc2 // 4
                            nc.tensor.matmul(
                                ps1[MSEG * band:MSEG * (band + 1),
                                    w2 * P:(w2 + 1) * P],
                                lhsT=Bt[:, crel * MSEG:(crel + 1) * MSEG],
                                rhs=msgs[b][:, ci, :],
                                start=True,
                                stop=True,
                                tile_position=(0, MSEG * band),
                            )
                        Pt = pP.tile([P, 4, OF], f16, tag="P")
                        nc.vector.tensor_copy(
                            Pt[:, :nw2, :].rearrange("p a b -> p (a b)"),
                            ps1[:, :nw2 * P])
                        pts.append(Pt)

                    # L2: one CONTIGUOUS matmul burst per dst tile (PSUM
                    # accumulation groups must not interleave within a bank)
                    # all TPG dst-tile accumulators packed into one bank
                    l2t = psL2.tile([P, TPG, OF], f32, tag="l2",
                                    name=f"l2ps_{g}")
                    for t in tiles_g:
                        my_mms = [i for i in range(mm0, mm0 + nmm)
                                  if int(pl.mm_t[i]) == t]
                        reg = l2t[:, t - g * TPG, :]
                        # bias: out_tile += deg_w (x) bias   (rank-1)
                        nc.tensor.matmul(
                            reg,
                            lhsT=dw2[:, t * P:(t + 1) * P],
                            rhs=b2[:],
                            start=True,
                            stop=not my_mms,
                        )
                        for q, i in enumerate(my_mms):
                            j = int(pl.mm_j[i])
                            nc.tensor.matmul(
                                reg,
                                lhsT=s2t[:, (i - mm0) * P:(i - mm0 + 1) * P],
                                rhs=pts[j // 4][:, j % 4, :],
                                start=False,
                                stop=(q == len(my_mms) - 1),
                            )

                    ntg = len(tiles_g)
                    t0 = tiles_g[0]
                    ot = pout.tile([P, TPG, OF], f32, tag="out")
                    nc.vector.tensor_copy(
                        ot[:, :ntg, :].rearrange("p a b -> p (a b)"),
                        l2t[:, :ntg, :].rearrange("p a b -> p (a b)"))
                    nc.sync.dma_start(
                        out_d[t0 * P:(t0 + ntg) * P, :].rearrange(
                            "(t p) f -> p t f", p=P),
                        ot[:, :ntg, :])

    nc.compile()
    return nc


# ---------------------------------------------------------------------------
# Entry point
# ---------------------------------------------------------------------------

def kernel(x, edge_src, edge_dst, edge_vals, weight, bias,
           _want_trace=False, _n_cores=None):
    x = np.asarray(x)
    edge_src = np.asarray(edge_src)
    edge_dst = np.asarray(edge_dst)
    edge_vals = np.asarray(edge_vals)
    weight = np.asarray(weight)
    bias = np.asarray(bias)

    pl = build_plan(x, edge_src, edge_dst, edge_vals, weight, bias)
    nc = build_bass(pl)

    from concourse.bass_utils import run_bass_kernel_spmd

    ncores = N_CORES if _n_cores is None else _n_cores
    in_maps = []
    for ci in range(ncores):
        in_maps.append({
            "xt": pl.xT,
            "w": pl.W,
            "bvec": pl.bvec,
            "degw": np.ascontiguousarray(pl.degw[ci]),
            "idx": np.ascontiguousarray(pl.IDX[ci]),
            "bmat": np.ascontiguousarray(pl.Bf[ci]),
            "s2": np.ascontiguousarray(pl.S2f[ci]),
        })
    res = run_bass_kernel_spmd(nc, in_maps, core_ids=list(range(ncores)),
                               trace=_want_trace)
    outs = [res.results[ci]["out"][:pl.ndst, :] for ci in range(ncores)]
    if ncores < N_CORES:
        outs += [np.zeros((pl.ndst, pl.OUT_F), np.float32)] * (
            N_CORES - ncores)
    full = np.concatenate(outs, axis=0).astype(np.float32)
    if _want_trace:
        kernel._last_results = res
    return full


# revision 14
# speedup vs baseline: 3.5204x; 1.0164x over previous
"""GCNConv kernel for 8x Trainium2 NeuronCores (Bass/Tile).

Reference computation:
    h = x @ W + b                  # [N, 256] @ [256, 128] -> [N, 128]
    out[i] = sum_{e: dst[e]=i} val[e] * h[src[e]]

Strategy (per core; SPMD - one program, per-core data):
  - dst nodes sharded 12500/core (output rows).  Edges partitioned by dst.
  - Phase 1: every core computes the full h (fp16) into 4 per-window DRAM
    tensors via PE matmuls (host passes x transposed + fp16), so phase-2
    gathers of window w can start as soon as window w is projected.
  - Bias handled as a rank-1 matmul per dst tile: out_tile += deg_w (x) bias
    (deg_w = per-dst sum of incident edge vals, host-precomputed).
  - Phase 2: per-edge h rows pulled on-chip with dma_gather (int16 indices,
    4 source windows of <=25088 rows), SWDGE queues cycled.  Per 128-edge
    chunk, a host-built staircase matrix B [128e x 32seg] fp16 (carrying
    val) is the stationary matmul operand -> PSUM partial segment sums.  A
    second host-built one-hot S2 [128seg x 128dst] matmul accumulates
    segments into per-dst-tile PSUM, which is written out fp32.
  - Chunks are packed at (group, window) granularity (crossing dst-tile
    boundaries) to minimize gather padding.
All data-dependent structure is padded to the max across cores so the same
program serves all 8 cores.
"""

import os
import sys

for _p in ("/opt/trn_rl_repo",):
    if _p not in sys.path:
        sys.path.insert(0, _p)

import numpy as np

P = 128
MSEG = 32            # segment slots per 128-edge chunk
TPG = 4              # dst tiles (of 128 dst) per processing group
N_CORES = 8
RB = 512             # projection row-batch


def _ceil_to(a, m):
    return -(-a // m) * m


class Plan:
    """Static (core-invariant) program structure + per-core data arrays."""


def build_plan(x, edge_src, edge_dst, edge_vals, weight, bias):
    N, IN_F = x.shape
    OUT_F = weight.shape[1]
    assert N % N_CORES == 0
    ndst = N // N_CORES                    # dst nodes per core
    ndst_pad = _ceil_to(ndst, P)
    ntile = ndst_pad // P                  # dst tiles per core
    ngrp = -(-ntile // TPG)
    hrows = _ceil_to(N, P)
    blkr = 25088                           # window rows (mult of 512, <32767)
    nblk = -(-hrows // blkr)
    wrows = [min(blkr, hrows - w * blkr) for w in range(nblk)]

    pl = Plan()
    pl.N, pl.IN_F, pl.OUT_F = N, IN_F, OUT_F
    pl.ndst, pl.ndst_pad, pl.ntile, pl.ngrp = ndst, ndst_pad, ntile, ngrp
    pl.hrows, pl.nblk, pl.blkr, pl.wrows = hrows, nblk, blkr, wrows
    pl.kc = IN_F // P
    pl.grp_tiles = [list(range(g * TPG, min((g + 1) * TPG, ntile)))
                    for g in range(ngrp)]

    # --- dense inputs ---
    import ml_dtypes
    f8 = ml_dtypes.float8_e4m3
    xf8 = os.environ.get("K_XF8", "0") == "1"
    xdt = f8 if xf8 else np.float16
    pl.xf8 = xf8
    xT = np.zeros((pl.kc, P, hrows), xdt)
    xT[:, :, :N] = np.ascontiguousarray(x.astype(np.float32).T).reshape(
        pl.kc, P, N).astype(xdt)
    pl.xT = xT
    pl.W = np.ascontiguousarray(
        weight.astype(np.float32).reshape(pl.kc, P, OUT_F).transpose(
            1, 0, 2)).astype(xdt)
    pl.bvec = np.ascontiguousarray(bias.astype(np.float16)[None, :])

    deg_w = np.bincount(edge_dst, weights=edge_vals.astype(np.float64),
                        minlength=N).astype(np.float32)
    degw = np.zeros((N_CORES, 1, ndst_pad), np.float16)
    degw[:, 0, :ndst] = deg_w.reshape(N_CORES, ndst).astype(np.float16)
    pl.degw = degw

    # --- edges, sorted for (core, grp, blk) streams ---
    src = edge_src.astype(np.int64)
    dst = edge_dst.astype(np.int64)
    val = edge_vals.astype(np.float32)
    E = len(src)
    core = dst // ndst
    dl = dst % ndst
    grp = (dl // P) // TPG
    blk = src // blkr
    order = np.lexsort((src, dl, blk, grp, core))
    src, dst, val = src[order], dst[order], val[order]
    core, dl, grp, blk = core[order], dl[order], grp[order], blk[order]

    nbuck = ngrp * nblk
    bucket = (core * ngrp + grp) * nblk + blk
    bcount = np.bincount(bucket, minlength=N_CORES * nbuck)
    bstart = np.concatenate([[0], np.cumsum(bcount)[:-1]])
    slot_in_b = np.arange(E) - bstart[bucket]

    run_change = np.ones(E, bool)
    run_change[1:] = (bucket[1:] != bucket[:-1]) | (dl[1:] != dl[:-1])

    # fast path: chunks are fixed 128-slot windows of each bucket stream
    def compute_cd(slot_in_b):
        chunk_loc = slot_in_b // P
        piece_flag = run_change | (slot_in_b % P == 0)
        piece_id = np.cumsum(piece_flag) - 1
        cs = np.where(slot_in_b % P == 0, piece_id, -1)
        first = np.maximum.accumulate(cs)
        d = piece_id - first
        return chunk_loc, piece_flag, d

    chunk_loc, piece_flag, d = compute_cd(slot_in_b)

    if d.max() >= MSEG:
        # rare: some 128-slot window has >MSEG runs; redo those buckets with
        # a greedy that closes chunks early (slot padding inside the bucket)
        bad = np.unique(bucket[d >= MSEG])
        for bb in bad:
            lo, n = bstart[bb], bcount[bb]
            hi = lo + n
            rc = run_change[lo:hi]
            starts = np.nonzero(rc)[0]
            lens = np.diff(np.append(starts, n))
            pos = np.empty(n, np.int64)
            c, s, dd = 0, 0, 0
            for st, ln in zip(starts, lens):
                rem, off = ln, 0
                while rem > 0:
                    if s == P or dd == MSEG:
                        c += 1
                        s, dd = 0, 0
                    take = min(P - s, rem)
                    pos[st + off:st + off + take] = c * P + s + np.arange(take)
                    s += take
                    dd += 1
                    rem -= take
                    off += take
            slot_in_b[lo:hi] = pos
        chunk_loc, piece_flag, d = compute_cd(slot_in_b)
        assert d.max() < MSEG

    # per-bucket chunk counts -> static maxima
    cc_b = np.zeros(N_CORES * nbuck, np.int64)
    has = bcount > 0
    last_idx = bstart + bcount - 1
    cc_b[has] = chunk_loc[last_idx[has]] + 1
    cc_cb = cc_b.reshape(N_CORES, ngrp, nblk)
    CH = cc_cb.max(axis=0)                 # [ngrp, nblk] static chunk counts
    CC_g = CH.sum(axis=1)                  # per-group chunks
    CC = int(CC_g.sum())
    TOT = CC * P

    choff = np.zeros((ngrp, nblk), np.int64)
    c_lo = np.zeros(ngrp + 1, np.int64)
    off = 0
    for g in range(ngrp):
        c_lo[g] = off
        for b in range(nblk):
            choff[g, b] = off
            off += int(CH[g, b])
    c_lo[ngrp] = off
    assert off == CC

    chunk_glob = choff[grp, blk] + chunk_loc
    slot_glob = choff[grp, blk] * P + slot_in_b

    # idx array (int16 window-local row), wrapped per 16, replicated x8
    idx_flat = np.zeros((N_CORES, TOT), np.int16)
    idx_flat[core, slot_glob] = (src - blk * blkr).astype(np.int16)
    IDX = np.ascontiguousarray(
        idx_flat.reshape(N_CORES, TOT // 16, 16).transpose(0, 2, 1))
    IDX = np.tile(IDX, (1, 8, 1))          # [N_CORES, 128, TOT // 16]
    pl.IDX = IDX

    # B staircase (vals)
    Bf = np.zeros((N_CORES, P, CC * MSEG), np.float16)
    Bf[core, slot_glob % P, chunk_glob * MSEG + d] = val.astype(np.float16)
    pl.Bf = Bf

    # segments (pieces) -> L2 one-hot S2
    pidx = np.nonzero(piece_flag)[0]
    p_core = core[pidx]
    p_g = grp[pidx]
    p_crel = chunk_glob[pidx] - c_lo[p_g]
    p_j = p_crel // 4
    p_band = p_crel % 4
    p_d = d[pidx]
    p_dl = dl[pidx]
    p_tile = p_dl // P                     # 0..ntile-1

    # program-static mm list: union over cores of (g, j, tile)
    key = (p_g * 100000 + p_j) * 1000 + p_tile
    ukey = np.unique(key)
    mm_g = ukey // 100000000
    mm_j = (ukey // 1000) % 100000
    mm_t = ukey % 1000
    NMM = len(ukey)
    mm_of_key = {int(k): i for i, k in enumerate(ukey)}
    p_mm = np.searchsorted(ukey, key)

    s2f8 = os.environ.get("K_S2F8", "0") == "1"
    S2f = np.zeros((N_CORES, P, NMM * P),
                   f8 if s2f8 else np.float16)
    S2f[p_core, p_band * MSEG + p_d, p_mm * P + (p_dl % P)] = 1.0
    pl.S2f = S2f
    pl.s2f8 = s2f8

    # per-(g,tile) mm bookkeeping for start/stop flags
    mm_stop = np.zeros(NMM, bool)
    seen = {}
    for i in range(NMM):
        seen[(int(mm_g[i]), int(mm_t[i]))] = i
    for (_, _), i in seen.items():
        mm_stop[i] = True
    # first mm index per group (columns of S2 are mm-ordered = (g, j, t))
    mm0_g = np.searchsorted(mm_g, np.arange(ngrp), side="left")
    nmm_g = np.searchsorted(mm_g, np.arange(ngrp), side="right") - mm0_g

    pl.CC, pl.TOT, pl.NMM = CC, TOT, NMM
    pl.CH, pl.CC_g, pl.choff, pl.c_lo = CH, CC_g, choff, c_lo
    pl.mm_g, pl.mm_j, pl.mm_t, pl.mm_stop = mm_g, mm_j, mm_t, mm_stop
    pl.mm0_g, pl.nmm_g = mm0_g, nmm_g
    return pl


# ---------------------------------------------------------------------------
# Device program
# ---------------------------------------------------------------------------

def build_bass(pl):
    import concourse.bass as bass
    import concourse.mybir as mybir
    import concourse.tile as tile
    from concourse import bacc

    f16 = mybir.dt.float16
    f32 = mybir.dt.float32
    f8 = mybir.dt.float8e4
    i16 = mybir.dt.int16

    NSWQ = int(os.environ.get("K_NSWQ", "4"))
    GMAX = int(os.environ.get("K_GMAX", "8192"))
    nc = bacc.Bacc("TRN2", target_bir_lowering=False, debug=False,
                   num_swdge_queues=NSWQ)

    OF = pl.OUT_F
    xdt = f8 if pl.xf8 else f16
    xT_d = nc.dram_tensor("xt", [pl.kc, P, pl.hrows], xdt,
                          kind="ExternalInput")
    W_d = nc.dram_tensor("w", [P, pl.kc, OF], xdt, kind="ExternalInput")
    b_d = nc.dram_tensor("bvec", [1, OF], f16, kind="ExternalInput")
    dw_d = nc.dram_tensor("degw", [1, pl.ndst_pad], f16,
                          kind="ExternalInput")
    idx_d = nc.dram_tensor("idx", [P, pl.TOT // 16], i16,
                           kind="ExternalInput")
    B_d = nc.dram_tensor("bmat", [P, pl.CC * MSEG], f16,
                         kind="ExternalInput")
    S2_d = nc.dram_tensor("s2", [P, pl.NMM * P],
                          f8 if pl.s2f8 else f16,
                          kind="ExternalInput")
    out_d = nc.dram_tensor("out", [pl.ndst_pad, OF], f32,
                           kind="ExternalOutput")
    h_ds = [nc.dram_tensor(f"hbuf{w}", [pl.wrows[w], OF], f16)
            for w in range(pl.nblk)]

    qctr = [0]

    def next_q():
        q = qctr[0] % NSWQ
        qctr[0] += 1
        return q

    with tile.TileContext(nc) as tc:
        with tc.tile_pool(name="pconst", bufs=1) as pconst:
            W_sb = pconst.tile([P, pl.kc, OF], xdt)
            nc.sync.dma_start(W_sb[:], W_d[:])
            # deg_w / bias padded to K=128 (row 0 live, rest zero) so the
            # rank-1 bias matmul uses the same PE tile config as the S2 mms
            dw2 = pconst.tile([P, pl.ndst_pad], f16)
            nc.gpsimd.memset(dw2[:], 0.0)
            nc.sync.dma_start(dw2[0:1, :], dw_d[:])
            b2 = pconst.tile([P, OF], f16)
            nc.gpsimd.memset(b2[:], 0.0)
            nc.sync.dma_start(b2[0:1, :], b_d[:])

            # ------------- Phase 1: h = x @ W (per window) -------------
            with (
                tc.tile_pool(name="pxt", bufs=3) as pxt,
                tc.tile_pool(name="phs", bufs=3) as phs,
                tc.tile_pool(name="ppsum", bufs=2, space="PSUM") as ppsum,
            ):
                for w in range(pl.nblk):
                    r0g = w * pl.blkr
                    for rb in range(-(-pl.wrows[w] // RB)):
                        r0 = rb * RB
                        nrows = min(RB, pl.wrows[w] - r0)
                        nch = nrows // P
                        xt = pxt.tile([P, pl.kc, RB], xdt, tag="xt")
                        nc.sync.dma_start(
                            xt[:, :, :nrows],
                            xT_d[:, :, r0g + r0:r0g + r0 + nrows].rearrange(
                                "k p c -> p k c"),
                        )
                        ps = ppsum.tile([P, RB], f32, tag="pj")
                        for rc in range(nch):
                            for k in range(pl.kc):
                                nc.tensor.matmul(
                                    ps[:, rc * P:(rc + 1) * P],
                                    lhsT=xt[:, k, rc * P:(rc + 1) * P],
                                    rhs=W_sb[:, k, :],
                                    start=(k == 0),
                                    stop=(k == pl.kc - 1),
                                )
                        hs = phs.tile([P, RB], f16, tag="hs")
                        nc.vector.tensor_copy(hs[:, :nrows], ps[:, :nrows])
                        nc.sync.dma_start(
                            h_ds[w][r0:r0 + nrows, :].rearrange(
                                "(c p) f -> p c f", p=P),
                            hs[:, :nrows].rearrange("p (c f) -> p c f", f=OF),
                        )

            # ---------------- Phase 2: gather + L1 + L2 ----------------
            with (
                tc.tile_pool(name="pidx", bufs=4) as pidx,
                tc.tile_pool(name="pmsg", bufs=8) as pmsg,
                tc.tile_pool(name="pB", bufs=2) as pB,
                tc.tile_pool(name="pS2", bufs=2) as pS2,
                tc.tile_pool(name="pP",
                             bufs=int(max(-(-pl.CC_g // 16))) + 2) as pP,
                tc.tile_pool(name="pout", bufs=2) as pout,
                tc.tile_pool(name="psL1", bufs=2, space="PSUM") as psL1,
                tc.tile_pool(name="psL2", bufs=2, space="PSUM") as psL2,
            ):
                for g in range(pl.ngrp):
                    tiles_g = pl.grp_tiles[g]
                    ccg = int(pl.CC_g[g])
                    clo = int(pl.c_lo[g])
                    # group idx tile + gathers per window
                    ixg = pidx.tile([P, ccg * 8], i16, tag="idx")
                    nc.sync.dma_start(
                        ixg[:], idx_d[:, clo * 8:(clo + ccg) * 8])
                    msgs = {}
                    for b in range(pl.nblk):
                        n = int(pl.CH[g, b]) * P
                        if n == 0:
                            continue
                        o = int(pl.choff[g, b]) * P     # global slot offset
                        ol = o - clo * P                # offset within group
                        mt = pmsg.tile([P, n // P, OF], f16, tag="msg")
                        for q0 in range(0, n, GMAX):
                            qn = min(GMAX, n - q0)
                            nc.gpsimd.dma_gather(
                                out_ap=mt[:, q0 // P:(q0 + qn) // P, :],
                                in_ap=h_ds[b][:, :],
                                idxs_ap=ixg[:, (ol + q0) // 16:
                                            (ol + q0 + qn) // 16],
                                num_idxs=qn,
                                num_idxs_reg=qn,
                                elem_size=OF,
                                single_packet=False,
                                queue_num=next_q(),
                            )
                        msgs[b] = mt

                    # B + S2 for the whole group
                    Bt = pB.tile([P, ccg * MSEG], f16, tag="B")
                    nc.sync.dma_start(
                        Bt[:], B_d[:, clo * MSEG:(clo + ccg) * MSEG])
                    mm0 = int(pl.mm0_g[g])
                    nmm = int(pl.nmm_g[g])
                    s2t = pS2.tile([P, max(nmm, 1) * P],
                   f8 if pl.s2f8 else f16, tag="s2")
                    if nmm:
                        nc.sync.dma_start(
                            s2t[:, :nmm * P],
                            S2_d[:, mm0 * P:(mm0 + nmm) * P])

                    # window of each chunk (static)
                    win_of = np.repeat(np.arange(pl.nblk),
                                       pl.CH[g]).astype(np.int64)

                    pts = []
                    n_pg = -(-ccg // 16)
                    for pg in range(n_pg):
                        cc0 = 16 * pg
                        nch = min(16, ccg - cc0)
                        nw2 = -(-nch // 4)
                        ps1 = psL1.tile([P, 16 * MSEG], f32, tag="ps1")
                        for cc2 in range(nch):
                            crel = cc0 + cc2
                            b = int(win_of[crel])
                            ci = clo + crel - int(pl.choff[g, b])
                            band = cc2 % 4
                            w2 = cc2 // 4
                            nc.tensor.matmul(
                                ps1[MSEG * band:MSEG * (band + 1),
                                    w2 * P:(w2 + 1) * P],
                                lhsT=Bt[:, crel * MSEG:(crel + 1) * MSEG],
                                rhs=msgs[b][:, ci, :],
                                start=True,
                                stop=True,
                                tile_position=(0, MSEG * band),
                            )
                        Pt = pP.tile([P, 4, OF], f16, tag="P")
                        nc.vector.tensor_copy(
                            Pt[:, :nw2, :].rearrange("p a b -> p (a b)"),
                            ps1[:, :nw2 * P])
                        pts.append(Pt)

                    # L2: one CONTIGUOUS matmul burst per dst tile (PSUM
                    # accumulation groups must not interleave within a bank)
                    # all TPG dst-tile accumulators packed into one bank
                    l2t = psL2.tile([P, TPG, OF], f32, tag="l2",
                                    name=f"l2ps_{g}")
                    for t in tiles_g:
                        my_mms = [i for i in range(mm0, mm0 + nmm)
                                  if int(pl.mm_t[i]) == t]
                        reg = l2t[:, t - g * TPG, :]
                        # bias: out_tile += deg_w (x) bias   (rank-1)
                        nc.tensor.matmul(
                            reg,
                            lhsT=dw2[:, t * P:(t + 1) * P],
                            rhs=b2[:],
                            start=True,
                            stop=not my_mms,
                        )
                        for q, i in enumerate(my_mms):
                            j = int(pl.mm_j[i])
                            nc.tensor.matmul(
                                reg,
                                lhsT=s2t[:, (i - mm0) * P:(i - mm0 + 1) * P],
                                rhs=pts[j // 4][:, j % 4, :],
                                start=False,
                                stop=(q == len(my_mms) - 1),
                            )

                    ntg = len(tiles_g)
                    t0 = tiles_g[0]
                    ot = pout.tile([P, TPG, OF], f32, tag="out")
                    nc.vector.tensor_copy(
                        ot[:, :ntg, :].rearrange("p a b -> p (a b)"),
                        l2t[:, :ntg, :].rearrange("p a b -> p (a b)"))
                    nc.sync.dma_start(
                        out_d[t0 * P:(t0 + ntg) * P, :].rearrange(
                            "(t p) f -> p t f", p=P),
                        ot[:, :ntg, :])

    nc.compile()
    return nc


# ---------------------------------------------------------------------------
# Entry point
# ---------------------------------------------------------------------------

def kernel(x, edge_src, edge_dst, edge_vals, weight, bias,
           _want_trace=False, _n_cores=None):
    x = np.asarray(x)
    edge_src = np.asarray(edge_src)
    edge_dst = np.asarray(edge_dst)
    edge_vals = np.asarray(edge_vals)
    weight = np.asarray(weight)
    bias = np.asarray(bias)

    pl = build_plan(x, edge_src, edge_dst, edge_vals, weight, bias)
    nc = build_bass(pl)

    from concourse.bass_utils import run_bass_kernel_spmd

    ncores = N_CORES if _n_cores is None else _n_cores
    in_maps = []
    for ci in range(ncores):
        in_maps.append({
            "xt": pl.xT,
            "w": pl.W,
            "bvec": pl.bvec,
            "degw": np.ascontiguousarray(pl.degw[ci]),
            "idx": np.ascontiguousarray(pl.IDX[ci]),
            "bmat": np.ascontiguousarray(pl.Bf[ci]),
            "s2": np.ascontiguousarray(pl.S2f[ci]),
        })
    res = run_bass_kernel_spmd(nc, in_maps, core_ids=list(range(ncores)),
                               trace=_want_trace)
    outs = [res.results[ci]["out"][:pl.ndst, :] for ci in range(ncores)]
    if ncores < N_CORES:
        outs += [np.zeros((pl.ndst, pl.OUT_F), np.float32)] * (
            N_CORES - ncores)
    full = np.concatenate(outs, axis=0).astype(np.float32)
    if _want_trace:
        kernel._last_results = res
    return full
